# revision 5
# baseline (speedup 1.0000x reference)
"""Causal self-attention for TRN2, 8 NeuronCores, DP(batch=4) x TP(2 head-groups).

Problem (hardcoded): B=4, T=2048, C=1024, H=16 heads, hd=64.
  qkv = x @ qkv_w.T + qkv_b ; per-head causal softmax(q k^T / 8) @ v ; out @ proj_w.T + proj_b

Core (b, g) handles batch b and head-group g (8 heads = 512 channels).
All inputs stream as bf16; per-core state is bf16 except the far-past
attention path which runs fp8e4 DoubleRow (2x PE rate):

  - qkv projections: bf16 matmuls (12 channel-tiles per 512-token block),
    DVE evictions with bias add.
  - attention per head-pair hp in the transposed domain (2 heads packed in
    128 partitions via tile_position rows): scoresT[tk,tq] bf16, K=64.
  - exp via ACT: off-diagonal chunks -> fp8 pair-tiles (for DoubleRow),
    diagonal chunks -> bf16; causal mask applied by zeroing the upper
    triangle of diagonal ex blocks on GPSIMD (affine_select).
  - attn@v: off-diagonal chunk PAIRS via fp8 DoubleRow (K=256/pass) with v
    split into fp8 hi (+ones col, M=65, stride-80 layout) and subnormal lo;
    diagonal chunks via bf16 [v|1] (M=65) matmuls. Softmax denominator
    accumulates in PSUM row 64 throughout.
  - normalize: DVE reciprocal + GPSIMD partition-broadcast + DVE multiply.
  - proj: bf16 over this core's 512 channels; host sums the 2 group
    partials per batch (bf16) and adds proj_b.

PSUM (8 banks): sc [128,1024] x2 (4) + po0/po1 [65,512] (2) + shared
512-wide ring x2 (2) for qkv-acc / v-transpose / proj.
"""

import numpy as np
import ml_dtypes

import concourse.mybir as mybir
import concourse.tile as tile
from concourse import bacc
from concourse.bass_utils import run_bass_kernel_spmd
from concourse.masks import make_identity

F32 = mybir.dt.float32
BF16 = mybir.dt.bfloat16
FP8 = mybir.dt.float8e4
ACT_F = mybir.ActivationFunctionType
ALU = mybir.AluOpType
DR = mybir.MatmulPerfMode.DoubleRow

B, T, C, H, HD = 4, 2048, 1024, 16, 64
NCORES = 8
P = 128
CH = 512           # channels per core (8 heads)
NHP = 4            # head-pairs per core
NPAIR = 6          # off-diag chunk pairs per hp (chunks 0..11)
SCL = 0.125

E4 = ml_dtypes.float8_e4m3
BF = ml_dtypes.bfloat16

_CACHED = {}


class _Ctx:
    pass


def _build():
    nc = bacc.Bacc("TRN2", target_bir_lowering=False, debug=False)

    xb_d = nc.dram_tensor("xb", [C, T], BF16, kind="ExternalInput").ap()
    wq_d = nc.dram_tensor("wq", [C, CH], BF16, kind="ExternalInput").ap()
    wk_d = nc.dram_tensor("wk", [C, CH], BF16, kind="ExternalInput").ap()
    wv_d = nc.dram_tensor("wv", [C, CH], BF16, kind="ExternalInput").ap()
    pw_d = nc.dram_tensor("pw", [CH, C], BF16, kind="ExternalInput").ap()
    qb_d = nc.dram_tensor("qb", [CH, 1], F32, kind="ExternalInput").ap()
    kb_d = nc.dram_tensor("kb", [CH, 1], F32, kind="ExternalInput").ap()
    vb_d = nc.dram_tensor("vb", [CH, 1], F32, kind="ExternalInput").ap()
    yT_d = nc.dram_tensor("yT", [C, T], BF16, kind="ExternalOutput").ap()

    with tile.TileContext(nc) as tc:
        with (
            tc.tile_pool(name="const", bufs=1) as pc,
            tc.tile_pool(name="xs", bufs=2) as px,
            tc.tile_pool(name="big", bufs=1) as pb,
            tc.tile_pool(name="exp", bufs=3) as pe,
            tc.tile_pool(name="small", bufs=4) as psm,
            tc.tile_pool(name="yev", bufs=6) as py,
            tc.tile_pool(name="ps", bufs=1, space="PSUM") as pp,
        ):
            # ---- constants ----
            identb = pc.tile([P, P], BF16, tag="identb")
            make_identity(nc, identb)
            ones8 = pc.tile([P, 1], FP8, tag="ones8")
            nc.vector.memset(ones8[:], 1.0)
            onesb = pc.tile([P, 1], BF16, tag="onesb")
            nc.vector.memset(onesb[:], 1.0)

            # ---- weights / biases ----
            wqt = pc.tile([P, 8, CH], BF16, tag="wqt")
            nc.sync.dma_start(wqt[:], wq_d.rearrange("(k p) c -> p k c", p=P))
            wkt = pc.tile([P, 8, CH], BF16, tag="wkt")
            nc.sync.dma_start(wkt[:], wk_d.rearrange("(k p) c -> p k c", p=P))
            wvt = pc.tile([P, 8, CH], BF16, tag="wvt")
            nc.sync.dma_start(wvt[:], wv_d.rearrange("(k p) c -> p k c", p=P))
            pwt = pc.tile([P, NHP, C], BF16, tag="pwt")
            nc.sync.dma_start(pwt[:], pw_d.rearrange("(a p) c -> p a c", p=P))
            qbt = pc.tile([P, NHP, 1], F32, tag="qbt")
            nc.sync.dma_start(qbt[:], qb_d.rearrange("(a p) o -> p a o", p=P))
            kbt = pc.tile([P, NHP, 1], F32, tag="kbt")
            nc.sync.dma_start(kbt[:], kb_d.rearrange("(a p) o -> p a o", p=P))
            vbt = pc.tile([P, NHP, 1], F32, tag="vbt")
            nc.sync.dma_start(vbt[:], vb_d.rearrange("(a p) o -> p a o", p=P))

            # ---- per-core state ----
            qTb = pb.tile([P, NHP, T], BF16, tag="qTb")
            kTb = pb.tile([P, NHP, T], BF16, tag="kTb")
            vT = pb.tile([P, NHP, T], BF16, tag="vT")
            attn = pb.tile([P, NHP, T], BF16, tag="attn")
            # diag v_sb: per (hp, l, ck): 64 v-cols + ones col (bf16, M=65)
            vsb_t = pb.tile([P, NHP * 2 * 16 * 65], BF16, tag="vsb")
            vsb = vsb_t.rearrange("p (h l ck c) -> p h l ck c", h=NHP, l=2, ck=16)
            # off-diag hi/lo fp8, DoubleRow layout: (hp, l, pair, kt, stride 80)
            vhi_t = pb.tile([P, NHP * 2 * NPAIR * 2 * 80], FP8, tag="vhi")
            vhi = vhi_t.rearrange("p (h l pr kt c) -> p h l pr kt c",
                                  h=NHP, l=2, pr=NPAIR, kt=2)
            vlo_t = pb.tile([P, NHP * 2 * NPAIR * 2 * 64], FP8, tag="vlo")
            vlo = vlo_t.rearrange("p (h l pr kt c) -> p h l pr kt c",
                                  h=NHP, l=2, pr=NPAIR, kt=2)
            # ones slots
            nc.vector.tensor_copy(
                vsb_t.rearrange("p (s c) -> p s c", c=65)[:, :, 64:65],
                onesb[:, None, 0:1].broadcast_to((P, NHP * 2 * 16, 1)))
            nc.vector.tensor_copy(
                vhi_t.rearrange("p (s c) -> p s c", c=80)[:, :, 64:65],
                ones8[:, None, 0:1].broadcast_to((P, NHP * 2 * NPAIR * 2, 1)))

            def qkv_gen():
                """1 + 4x12 = 49 units."""
                def load_x(nbq):
                    blk = slice(nbq * 512, nbq * 512 + 512)
                    xbt = px.tile([P, 8, 512], BF16, tag="xbt",
                                  name=f"xbt_{nbq}")
                    nc.sync.dma_start(
                        xbt[:],
                        xb_d.rearrange("(k p) t -> p k t", p=P)[:, :, blk])
                    return xbt

                xnext = load_x(0)
                yield
                for nbq in range(4):
                    xbt = xnext
                    if nbq + 1 < 4:
                        xnext = load_x(nbq + 1)
                    blk = slice(nbq * 512, (nbq + 1) * 512)
                    for wt, dst, bias in ((wvt, vT, vbt), (wkt, kTb, kbt),
                                          (wqt, qTb, qbt)):
                        for m in range(NHP):
                            acc = pp.tile([P, 512], F32, tag="w512", bufs=2,
                                          name=f"acc_{nbq}_{m}")
                            for kt in range(8):
                                nc.tensor.matmul(
                                    acc[:], wt[:, kt, P * m:P * (m + 1)],
                                    xbt[:, kt, :],
                                    start=(kt == 0), stop=(kt == 7))
                            nc.vector.tensor_scalar(
                                out=dst[:, m, blk], in0=acc[:],
                                scalar1=bias[:, m, 0:1], scalar2=None,
                                op0=ALU.add)
                            yield

            def vsb_gen():
                """64 transposes, (nbq, hp, ck) order."""
                for nbq in range(4):
                    for hp in range(NHP):
                        for ck in range(4 * nbq, 4 * nbq + 4):
                            tp = pp.tile([P, P], BF16, tag="w512", bufs=2,
                                         name=f"tp_{hp}_{ck}")
                            nc.tensor.transpose(
                                tp[:], vT[:, hp, ck * P:(ck + 1) * P],
                                identb[:])
                            src = tp[:].rearrange("p (l c) -> p l c", l=2)
                            nc.vector.tensor_copy(
                                vsb[:, hp, :, ck, 0:64], src)
                            if ck < 2 * NPAIR:
                                pair, kt = ck // 2, ck % 2
                                dst8 = vhi[:, hp, :, pair, kt, 0:64]
                                nc.vector.tensor_copy(dst8, src)
                                nc.vector.tensor_tensor(
                                    out=vlo[:, hp, :, pair, kt, :],
                                    in0=src, in1=dst8, op=ALU.subtract)
                            yield

            def attn_gen(ready):
                for jj in range(4):
                    tqa = jj * 512
                    for hp in range(NHP):
                        po = [pp.tile([65, 512], F32, tag=f"po{l}", bufs=1,
                                      name=f"po_{jj}_{hp}_{l}")
                              for l in range(2)]
                        nch = 4 * jj + 4
                        first = {"f": True}
                        pend = []

                        def emit_vmm(item):
                            kind, a, ex = item
                            st = first["f"]
                            first["f"] = False
                            if kind == "pair":
                                for l in range(2):
                                    rhs = ex[:, :, 512 * l:512 * (l + 1)]
                                    nc.tensor.matmul(
                                        po[l][:, :],
                                        vhi[:, hp, l, a, :, 0:65], rhs,
                                        start=st, stop=False,
                                        perf_mode=DR, skip_group_check=True)
                                    nc.tensor.matmul(
                                        po[l][0:64, :],
                                        vlo[:, hp, l, a, :, :], rhs,
                                        start=False, stop=False,
                                        perf_mode=DR, skip_group_check=True)
                            else:
                                ck, w = a
                                last = ck == nch - 1
                                for l in range(2):
                                    nc.tensor.matmul(
                                        po[l][:, w:512],
                                        vsb[:, hp, l, ck, 0:65],
                                        ex[:, 512 * l + w:512 * (l + 1)],
                                        start=st, stop=last,
                                        skip_group_check=True)

                        ex8 = None
                        for ck in range(nch):
                            w = max(0, 128 * (ck - 4 * jj))
                            diag = ck >= 4 * jj
                            sc = pp.tile([P, 1024], F32, tag="sc", bufs=2,
                                         name=f"sc_{jj}_{hp}_{ck}")
                            for l in range(2):
                                nc.tensor.matmul(
                                    sc[:, 512 * l + w:512 * (l + 1)],
                                    kTb[64 * l:64 * l + 64, hp,
                                        ck * P:(ck + 1) * P],
                                    qTb[64 * l:64 * l + 64, hp,
                                        tqa + w:tqa + 512],
                                    start=True, stop=True,
                                    tile_position=(64 * l, 0))
                            if diag:
                                exb = pe.tile([P, 1024], BF16, tag="exb",
                                              bufs=3, name=f"exb_{jj}_{hp}_{ck}")
                                nc.scalar.activation(
                                    exb[:, w:1024], sc[:, w:1024],
                                    ACT_F.Exp, scale=SCL)
                                exv = exb[:].rearrange(
                                    "p (l t) -> p l t", l=2)[:, :, w:w + P]
                                nc.gpsimd.affine_select(
                                    out=exv, in_=exv,
                                    compare_op=ALU.is_ge, fill=0.0, base=0,
                                    pattern=[[0, 2], [1, P]],
                                    channel_multiplier=-1)
                                pend.append(("diag", (ck, w), exb))
                            else:
                                par = ck % 2
                                if par == 0:
                                    ex8 = pe.tile([P, 2, 1024], FP8,
                                                  tag="ex8", bufs=3,
                                                  name=f"ex8_{jj}_{hp}_{ck}")
                                nc.scalar.activation(
                                    ex8[:, par, :], sc[:, :],
                                    ACT_F.Exp, scale=SCL)
                                if par == 1:
                                    pend.append(("pair", ck // 2, ex8))
                            if len(pend) > 1:
                                emit_vmm(pend.pop(0))
                            yield
                        while pend:
                            emit_vmm(pend.pop(0))
                        for l in range(2):
                            rc = psm.tile([1, 512], F32, tag="rc", bufs=4,
                                          name=f"rc_{jj}_{hp}_{l}")
                            nc.vector.reciprocal(rc[:], po[l][64:65, :])
                            rb = psm.tile([64, 512], F32, tag="rb", bufs=4,
                                          name=f"rb_{jj}_{hp}_{l}")
                            nc.gpsimd.partition_broadcast(rb[:], rc[0:1, :])
                            nc.vector.tensor_tensor(
                                out=attn[64 * l:64 * l + 64, hp,
                                         tqa:tqa + 512],
                                in0=po[l][0:64, :], in1=rb[:], op=ALU.mult)
                        yield
                    ready.append(jj)

            def proj_gen(ready):
                done = 0
                while done < 4:
                    if not ready:
                        yield
                        continue
                    nb = ready.pop(0)
                    blk = slice(nb * 512, (nb + 1) * 512)
                    for ob in range(8):
                        pj = pp.tile([P, 512], F32, tag="w512", bufs=2,
                                     name=f"pj_{nb}_{ob}")
                        for hp in range(NHP):
                            nc.tensor.matmul(
                                pj[:], pwt[:, hp, ob * P:(ob + 1) * P],
                                attn[:, hp, blk],
                                start=(hp == 0), stop=(hp == 3))
                        ysb = py.tile([P, 512], BF16, tag="ysb",
                                      name=f"ysb_{nb}_{ob}")
                        nc.vector.tensor_copy(ysb[:], pj[:])
                        nc.sync.dma_start(
                            yT_d[ob * P:(ob + 1) * P, blk], ysb[:])
                        yield
                    done += 1

            # ---- software pipeline ----
            qk = qkv_gen()
            vs = vsb_gen()
            ready = []
            at = attn_gen(ready)
            pj = proj_gen(ready)

            nqk, nvs = [0], [0]
            _DONE = object()

            def adv(gen, cnt, target):
                while cnt[0] < target:
                    if next(gen, _DONE) is _DONE:
                        break
                    cnt[0] += 1

            # prologue: nbq0 qkv (1 load + 12 units) + first 4 transposes
            adv(qk, nqk, 13)
            adv(vs, nvs, 4)

            # piecewise-linear emission targets over the 176 attn yields
            bnd = [0, 20, 56, 108, 176]
            qk_tgt = [13, 25, 37, 49, 49]
            vs_tgt = [4, 32, 48, 64, 64]

            def interp(i, tgt):
                for s in range(4):
                    if i < bnd[s + 1]:
                        f = (i - bnd[s]) / (bnd[s + 1] - bnd[s])
                        return int(tgt[s] + (tgt[s + 1] - tgt[s]) * f + 0.999)
                return tgt[4]

            i = 0
            for _ in at:
                adv(qk, nqk, interp(i, qk_tgt))
                adv(vs, nvs, interp(i, vs_tgt))
                if ready and i % 4 == 0:
                    next(pj, None)
                i += 1
            for gen in (qk, vs, pj):
                for _ in gen:
                    pass

    nc.compile()
    return nc


def get_nc():
    if "nc" not in _CACHED:
        _CACHED["nc"] = _build()
    return _CACHED["nc"]


def kernel(x, qkv_w, qkv_b, proj_w, proj_b):
    x = np.asarray(x, dtype=np.float32)
    qkv_w = np.asarray(qkv_w, dtype=np.float32)
    qkv_b = np.asarray(qkv_b, dtype=np.float32)
    proj_w = np.asarray(proj_w, dtype=np.float32)
    proj_b = np.asarray(proj_b, dtype=np.float32)

    in_maps = []
    for c in range(NCORES):
        b, grp = c // 2, c % 2
        cs = slice(CH * grp, CH * (grp + 1))
        xT = np.ascontiguousarray(x[b].T)              # [C, T]
        in_maps.append({
            "xb": xT.astype(BF),
            "wq": np.ascontiguousarray(qkv_w[cs, :].T).astype(BF),
            "wk": np.ascontiguousarray(qkv_w[C:][cs, :].T).astype(BF),
            "wv": np.ascontiguousarray(qkv_w[2 * C:][cs, :].T).astype(BF),
            "pw": np.ascontiguousarray(proj_w[:, cs].T).astype(BF),
            "qb": qkv_b[cs].reshape(CH, 1).astype(np.float32),
            "kb": qkv_b[C:][cs].reshape(CH, 1).astype(np.float32),
            "vb": qkv_b[2 * C:][cs].reshape(CH, 1).astype(np.float32),
        })

    nc = get_nc()
    res = run_bass_kernel_spmd(nc, in_maps, list(range(NCORES)))

    y = np.empty((B, T, C), dtype=np.float32)
    pbf = proj_b.astype(np.float64)[None, :]
    for b in range(B):
        yT = (res.results[2 * b]["yT"].astype(np.float64)
              + res.results[2 * b + 1]["yT"].astype(np.float64))
        y[b] = (yT.T + pbf).astype(np.float32)
    return y


# revision 6
# speedup vs baseline: 1.1058x; 1.1058x over previous
"""Causal self-attention for TRN2, 8 NeuronCores, DP(batch=4) x TP(2 head-groups).

Problem (hardcoded): B=4, T=2048, C=1024, H=16 heads, hd=64.
  qkv = x @ qkv_w.T + qkv_b ; per-head causal softmax(q k^T / 8) @ v ; out @ proj_w.T + proj_b

Core (b, g) handles batch b and head-group g (8 heads = 512 channels).
All inputs stream as bf16; the far-past attention path runs fp8e4
DoubleRow (2x PE rate):

  - qkv projections: bf16 matmuls, q/k evicted via ACT Identity+bias,
    v via DVE.
  - attention per head-pair hp in the transposed domain (2 heads packed in
    128 partitions via tile_position rows): scoresT[tk,tq] bf16, K=64.
  - exp via ACT: off-diagonal chunks -> fp8 pair-tiles (for DoubleRow),
    diagonal chunks -> bf16; causal mask by zeroing the upper triangle of
    diagonal ex blocks on GPSIMD (affine_select).
  - attn@v: off-diagonal chunk PAIRS via fp8 DoubleRow (K=256/pass) with v
    split into fp8 hi (+ones col, M=65, stride-80 layout) and subnormal lo;
    diagonal chunks via bf16 [v|1] (M=65). Softmax denominator accumulates
    in PSUM row 64 throughout.
  - normalize: DVE reciprocal + GPSIMD partition-broadcast + DVE multiply.
  - proj: bf16 over this core's 512 channels; host sums the 2 group
    partials per batch (bf16) and adds proj_b.

Emission order is the schedule (in-order engine queues): qkv / v-transpose
/ proj units are paced by per-unit deadlines derived from when the
attention loop first needs their outputs, keeping filler work available
through the exp-latency-bound chunk loop (including late jj blocks).

PSUM (8 banks): sc [128,1024] x2 (4) + po0/po1 [65,512] (2) + shared
512-wide ring x2 (2) for qkv-acc / v-transpose / proj.
"""

import numpy as np
import ml_dtypes

import concourse.mybir as mybir
import concourse.tile as tile
from concourse import bacc
from concourse.bass_utils import run_bass_kernel_spmd
from concourse.masks import make_identity

F32 = mybir.dt.float32
BF16 = mybir.dt.bfloat16
FP8 = mybir.dt.float8e4
ACT_F = mybir.ActivationFunctionType
ALU = mybir.AluOpType
DR = mybir.MatmulPerfMode.DoubleRow

B, T, C, H, HD = 4, 2048, 1024, 16, 64
NCORES = 8
P = 128
CH = 512           # channels per core (8 heads)
NHP = 4            # head-pairs per core
NPAIR = 6          # off-diag chunk pairs per hp (chunks 0..11)
SCL = 0.125

E4 = ml_dtypes.float8_e4m3
BF = ml_dtypes.bfloat16

_CACHED = {}

# attn yield-index layout: jj block starts / per-hp strides
_S = [0, 20, 56, 108]
_STRIDE = [5, 9, 13, 17]
_NYIELD = 176


def _build():
    nc = bacc.Bacc("TRN2", target_bir_lowering=False, debug=False)

    xb_d = nc.dram_tensor("xb", [C, T], BF16, kind="ExternalInput").ap()
    wq_d = nc.dram_tensor("wq", [C, CH], BF16, kind="ExternalInput").ap()
    wk_d = nc.dram_tensor("wk", [C, CH], BF16, kind="ExternalInput").ap()
    wv_d = nc.dram_tensor("wv", [C, CH], BF16, kind="ExternalInput").ap()
    pw_d = nc.dram_tensor("pw", [CH, C], BF16, kind="ExternalInput").ap()
    qb_d = nc.dram_tensor("qb", [CH, 1], F32, kind="ExternalInput").ap()
    kb_d = nc.dram_tensor("kb", [CH, 1], F32, kind="ExternalInput").ap()
    vb_d = nc.dram_tensor("vb", [CH, 1], F32, kind="ExternalInput").ap()
    yT_d = nc.dram_tensor("yT", [C, T], BF16, kind="ExternalOutput").ap()

    xb_r = xb_d.rearrange("(k p) t -> p k t", p=P)

    with tile.TileContext(nc) as tc:
        with (
            tc.tile_pool(name="const", bufs=1) as pc,
            tc.tile_pool(name="xs", bufs=2) as px,
            tc.tile_pool(name="big", bufs=1) as pb,
            tc.tile_pool(name="exp", bufs=3) as pe,
            tc.tile_pool(name="small", bufs=4) as psm,
            tc.tile_pool(name="yev", bufs=6) as py,
            tc.tile_pool(name="ps", bufs=1, space="PSUM") as pp,
        ):
            # ---- x block 0 + v weights first: first matmul depends on them
            xbt0 = px.tile([P, 8, 512], BF16, tag="xbt", name="xbt_0")
            nc.sync.dma_start(xbt0[:], xb_r[:, :, 0:512])
            wvt = pc.tile([P, 8, CH], BF16, tag="wvt")
            nc.sync.dma_start(wvt[:], wv_d.rearrange("(k p) c -> p k c", p=P))
            wkt = pc.tile([P, 8, CH], BF16, tag="wkt")
            nc.sync.dma_start(wkt[:], wk_d.rearrange("(k p) c -> p k c", p=P))
            wqt = pc.tile([P, 8, CH], BF16, tag="wqt")
            nc.sync.dma_start(wqt[:], wq_d.rearrange("(k p) c -> p k c", p=P))
            qbt = pc.tile([P, NHP, 1], F32, tag="qbt")
            nc.sync.dma_start(qbt[:], qb_d.rearrange("(a p) o -> p a o", p=P))
            kbt = pc.tile([P, NHP, 1], F32, tag="kbt")
            nc.sync.dma_start(kbt[:], kb_d.rearrange("(a p) o -> p a o", p=P))
            vbt = pc.tile([P, NHP, 1], F32, tag="vbt")
            nc.sync.dma_start(vbt[:], vb_d.rearrange("(a p) o -> p a o", p=P))
            pwt = pc.tile([P, NHP, C], BF16, tag="pwt")
            nc.sync.dma_start(pwt[:], pw_d.rearrange("(a p) c -> p a c", p=P))

            # ---- constants ----
            identb = pc.tile([P, P], BF16, tag="identb")
            make_identity(nc, identb)
            ones8 = pc.tile([P, 1], FP8, tag="ones8")
            nc.vector.memset(ones8[:], 1.0)
            onesb = pc.tile([P, 1], BF16, tag="onesb")
            nc.vector.memset(onesb[:], 1.0)

            # ---- per-core state ----
            qTb = pb.tile([P, NHP, T], BF16, tag="qTb")
            kTb = pb.tile([P, NHP, T], BF16, tag="kTb")
            vT = pb.tile([P, NHP, T], BF16, tag="vT")
            attn = pb.tile([P, NHP, T], BF16, tag="attn")
            vsb_t = pb.tile([P, NHP * 2 * 16 * 65], BF16, tag="vsb")
            vsb = vsb_t.rearrange("p (h l ck c) -> p h l ck c",
                                  h=NHP, l=2, ck=16)
            vhi_t = pb.tile([P, NHP * 2 * NPAIR * 2 * 80], FP8, tag="vhi")
            vhi = vhi_t.rearrange("p (h l pr kt c) -> p h l pr kt c",
                                  h=NHP, l=2, pr=NPAIR, kt=2)
            vlo_t = pb.tile([P, NHP * 2 * NPAIR * 2 * 64], FP8, tag="vlo")
            vlo = vlo_t.rearrange("p (h l pr kt c) -> p h l pr kt c",
                                  h=NHP, l=2, pr=NPAIR, kt=2)
            nc.vector.tensor_copy(
                vsb_t.rearrange("p (s c) -> p s c", c=65)[:, :, 64:65],
                onesb[:, None, 0:1].broadcast_to((P, NHP * 2 * 16, 1)))
            nc.vector.tensor_copy(
                vhi_t.rearrange("p (s c) -> p s c", c=80)[:, :, 64:65],
                ones8[:, None, 0:1].broadcast_to((P, NHP * 2 * NPAIR * 2, 1)))

            def qkv_gen():
                """48 units: per nbq, 4 v then (k,q) per hp."""
                xnext = xbt0
                for nbq in range(4):
                    xbt = xnext
                    if nbq + 1 < 4:
                        blk1 = slice((nbq + 1) * 512, (nbq + 2) * 512)
                        xnext = px.tile([P, 8, 512], BF16, tag="xbt",
                                        name=f"xbt_{nbq + 1}")
                        nc.sync.dma_start(xnext[:], xb_r[:, :, blk1])
                    blk = slice(nbq * 512, (nbq + 1) * 512)

                    def chain(wt, m):
                        acc = pp.tile([P, 512], F32, tag="w512", bufs=2,
                                      name=f"acc_{nbq}_{m}")
                        for kt in range(8):
                            nc.tensor.matmul(
                                acc[:], wt[:, kt, P * m:P * (m + 1)],
                                xbt[:, kt, :], start=(kt == 0), stop=(kt == 7))
                        return acc

                    for m in range(NHP):
                        acc = chain(wvt, m)
                        nc.vector.tensor_scalar(
                            out=vT[:, m, blk], in0=acc[:],
                            scalar1=vbt[:, m, 0:1], scalar2=None, op0=ALU.add)
                        yield
                    for m in range(NHP):
                        acc = chain(wkt, m)
                        nc.scalar.activation(
                            kTb[:, m, blk], acc[:], ACT_F.Identity,
                            bias=kbt[:, m, 0:1], scale=1.0)
                        yield
                        acc = chain(wqt, m)
                        nc.scalar.activation(
                            qTb[:, m, blk], acc[:], ACT_F.Identity,
                            bias=qbt[:, m, 0:1], scale=1.0)
                        yield

            def vsb_gen():
                """64 units: (nbq, hp, ck)."""
                for nbq in range(4):
                    for hp in range(NHP):
                        for ck in range(4 * nbq, 4 * nbq + 4):
                            tp = pp.tile([P, P], BF16, tag="w512", bufs=2,
                                         name=f"tp_{hp}_{ck}")
                            nc.tensor.transpose(
                                tp[:], vT[:, hp, ck * P:(ck + 1) * P],
                                identb[:])
                            src = tp[:].rearrange("p (l c) -> p l c", l=2)
                            dstb = vsb[:, hp, :, ck, 0:64]
                            nc.vector.tensor_copy(dstb, src)
                            if ck < 2 * NPAIR:
                                pair, kt = ck // 2, ck % 2
                                dst8 = vhi[:, hp, :, pair, kt, 0:64]
                                nc.vector.tensor_copy(dst8, dstb)
                                nc.vector.tensor_tensor(
                                    out=vlo[:, hp, :, pair, kt, :],
                                    in0=dstb, in1=dst8, op=ALU.subtract)
                            yield

            def attn_gen(ready):
                for jj in range(4):
                    tqa = jj * 512
                    for hp in range(NHP):
                        po = [pp.tile([65, 512], F32, tag=f"po{l}", bufs=1,
                                      name=f"po_{jj}_{hp}_{l}")
                              for l in range(2)]
                        nch = 4 * jj + 4
                        first = {"f": True}
                        pend = []

                        def emit_vmm(item, po=po, hp=hp, nch=nch, first=first):
                            kind, a, ex = item
                            st = first["f"]
                            first["f"] = False
                            if kind == "pair":
                                for l in range(2):
                                    rhs = ex[:, :, 512 * l:512 * (l + 1)]
                                    nc.tensor.matmul(
                                        po[l][:, :],
                                        vhi[:, hp, l, a, :, 0:65], rhs,
                                        start=st, stop=False,
                                        perf_mode=DR, skip_group_check=True)
                                    nc.tensor.matmul(
                                        po[l][0:64, :],
                                        vlo[:, hp, l, a, :, :], rhs,
                                        start=False, stop=False,
                                        perf_mode=DR, skip_group_check=True)
                            else:
                                ck, w = a
                                last = ck == nch - 1
                                for l in range(2):
                                    nc.tensor.matmul(
                                        po[l][:, w:512],
                                        vsb[:, hp, l, ck, 0:65],
                                        ex[:, 512 * l + w:512 * (l + 1)],
                                        start=st, stop=last,
                                        skip_group_check=True)

                        ex8 = None
                        for ck in range(nch):
                            w = max(0, 128 * (ck - 4 * jj))
                            diag = ck >= 4 * jj
                            sc = pp.tile([P, 1024], F32, tag="sc", bufs=2,
                                         name=f"sc_{jj}_{hp}_{ck}")
                            for l in range(2):
                                nc.tensor.matmul(
                                    sc[:, 512 * l + w:512 * (l + 1)],
                                    kTb[64 * l:64 * l + 64, hp,
                                        ck * P:(ck + 1) * P],
                                    qTb[64 * l:64 * l + 64, hp,
                                        tqa + w:tqa + 512],
                                    start=True, stop=True,
                                    tile_position=(64 * l, 0))
                            if diag:
                                exb = pe.tile([P, 1024], BF16, tag="exb",
                                              bufs=3,
                                              name=f"exb_{jj}_{hp}_{ck}")
                                nc.scalar.activation(
                                    exb[:, w:1024], sc[:, w:1024],
                                    ACT_F.Exp, scale=SCL)
                                exv = exb[:].rearrange(
                                    "p (l t) -> p l t", l=2)[:, :, w:w + P]
                                nc.gpsimd.affine_select(
                                    out=exv, in_=exv,
                                    compare_op=ALU.is_ge, fill=0.0, base=0,
                                    pattern=[[0, 2], [1, P]],
                                    channel_multiplier=-1)
                                pend.append(("diag", (ck, w), exb))
                            else:
                                par = ck % 2
                                if par == 0:
                                    ex8 = pe.tile([P, 2, 1024], FP8,
                                                  tag="ex8", bufs=3,
                                                  name=f"ex8_{jj}_{hp}_{ck}")
                                nc.scalar.activation(
                                    ex8[:, par, :], sc[:, :],
                                    ACT_F.Exp, scale=SCL)
                                if par == 1:
                                    pend.append(("pair", ck // 2, ex8))
                            if len(pend) > 1:
                                emit_vmm(pend.pop(0))
                            yield
                        while pend:
                            emit_vmm(pend.pop(0))
                        for l in range(2):
                            rc = psm.tile([1, 512], F32, tag="rc", bufs=4,
                                          name=f"rc_{jj}_{hp}_{l}")
                            nc.vector.reciprocal(rc[:], po[l][64:65, :])
                            rb = psm.tile([64, 512], F32, tag="rb", bufs=4,
                                          name=f"rb_{jj}_{hp}_{l}")
                            nc.gpsimd.partition_broadcast(rb[:], rc[0:1, :])
                            nc.vector.tensor_tensor(
                                out=attn[64 * l:64 * l + 64, hp,
                                         tqa:tqa + 512],
                                in0=po[l][0:64, :], in1=rb[:], op=ALU.mult)
                        yield
                    ready.append(jj)

            def proj_gen(ready):
                done = 0
                while done < 4:
                    if not ready:
                        yield
                        continue
                    nb = ready.pop(0)
                    blk = slice(nb * 512, (nb + 1) * 512)
                    for ob in range(8):
                        pj = pp.tile([P, 512], F32, tag="w512", bufs=2,
                                     name=f"pj_{nb}_{ob}")
                        for hp in range(NHP):
                            nc.tensor.matmul(
                                pj[:], pwt[:, hp, ob * P:(ob + 1) * P],
                                attn[:, hp, blk],
                                start=(hp == 0), stop=(hp == 3))
                        ysb = py.tile([P, 512], BF16, tag="ysb",
                                      name=f"ysb_{nb}_{ob}")
                        nc.vector.tensor_copy(ysb[:], pj[:])
                        nc.sync.dma_start(
                            yT_d[ob * P:(ob + 1) * P, blk], ysb[:])
                        yield
                    done += 1

            # ---- deadline-based pacing --------------------------------
            # qkv unit u (0-based, 12 per nbq: v x4 then k,q per hp):
            qk_dl = []
            for nbq in range(4):
                for m in range(NHP):       # v tiles: before vsb group nbq
                    qk_dl.append(max(0, _S[nbq] - 8))
                for m in range(NHP):       # k,q per hp: before attn(jj=nbq, hp=m)
                    d = _S[nbq] + m * _STRIDE[nbq]
                    qk_dl.extend([d, d])
            vs_dl = []
            for nbq in range(4):
                for hp in range(NHP):
                    d = _S[nbq] + hp * _STRIDE[nbq]
                    vs_dl.extend([d] * 4)

            LOOKAHEAD = 7

            def target(dl, i):
                t = 0
                for j, d in enumerate(dl):
                    if d <= i + LOOKAHEAD:
                        t = j + 1
                return t

            qk = qkv_gen()
            vs = vsb_gen()
            ready = []
            at = attn_gen(ready)
            pj = proj_gen(ready)

            nqk, nvs = [0], [0]
            _DONE = object()

            def adv(gen, cnt, tgt):
                while cnt[0] < tgt:
                    if next(gen, _DONE) is _DONE:
                        break
                    cnt[0] += 1

            adv(qk, nqk, 12)   # nbq0 fully
            adv(vs, nvs, 4)

            i = 0
            for _ in at:
                adv(qk, nqk, target(qk_dl, i))
                adv(vs, nvs, target(vs_dl, i))
                if ready and i % 6 == 0:
                    next(pj, None)
                i += 1
            for gen in (qk, vs, pj):
                for _ in gen:
                    pass

    nc.compile()
    return nc


def get_nc():
    if "nc" not in _CACHED:
        _CACHED["nc"] = _build()
    return _CACHED["nc"]


def kernel(x, qkv_w, qkv_b, proj_w, proj_b):
    x = np.asarray(x, dtype=np.float32)
    qkv_w = np.asarray(qkv_w, dtype=np.float32)
    qkv_b = np.asarray(qkv_b, dtype=np.float32)
    proj_w = np.asarray(proj_w, dtype=np.float32)
    proj_b = np.asarray(proj_b, dtype=np.float32)

    in_maps = []
    for c in range(NCORES):
        b, grp = c // 2, c % 2
        cs = slice(CH * grp, CH * (grp + 1))
        xT = np.ascontiguousarray(x[b].T)              # [C, T]
        in_maps.append({
            "xb": xT.astype(BF),
            "wq": np.ascontiguousarray(qkv_w[cs, :].T).astype(BF),
            "wk": np.ascontiguousarray(qkv_w[C:][cs, :].T).astype(BF),
            "wv": np.ascontiguousarray(qkv_w[2 * C:][cs, :].T).astype(BF),
            "pw": np.ascontiguousarray(proj_w[:, cs].T).astype(BF),
            "qb": qkv_b[cs].reshape(CH, 1).astype(np.float32),
            "kb": qkv_b[C:][cs].reshape(CH, 1).astype(np.float32),
            "vb": qkv_b[2 * C:][cs].reshape(CH, 1).astype(np.float32),
        })

    nc = get_nc()
    res = run_bass_kernel_spmd(nc, in_maps, list(range(NCORES)))

    y = np.empty((B, T, C), dtype=np.float32)
    pbf = proj_b.astype(np.float64)[None, :]
    for b in range(B):
        yT = (res.results[2 * b]["yT"].astype(np.float64)
              + res.results[2 * b + 1]["yT"].astype(np.float64))
        y[b] = (yT.T + pbf).astype(np.float32)
    return y


# revision 10
# speedup vs baseline: 1.1333x; 1.0249x over previous
"""Causal self-attention for TRN2, 8 NeuronCores, DP(batch=4) x TP(2 head-groups).

Problem (hardcoded): B=4, T=2048, C=1024, H=16 heads, hd=64.
  qkv = x @ qkv_w.T + qkv_b ; per-head causal softmax(q k^T / 8) @ v ; out @ proj_w.T + proj_b

Core (b, g) handles batch b and head-group g (8 heads = 512 channels).
All inputs stream as bf16; the far-past attention path runs fp8e4
DoubleRow (2x PE rate):

  - qkv projections: bf16 matmuls, q/k evicted via ACT Identity+bias,
    v via DVE.
  - attention per head-pair hp in the transposed domain (2 heads packed in
    128 partitions via tile_position rows): scoresT[tk,tq] bf16, K=64.
  - exp via ACT: off-diagonal chunks -> fp8 pair-tiles (for DoubleRow),
    diagonal chunks -> bf16; causal mask by zeroing the upper triangle of
    diagonal ex blocks on GPSIMD (affine_select).
  - attn@v: off-diagonal chunk PAIRS via fp8 DoubleRow (K=256/pass) with v
    split into fp8 hi (+ones col, M=65, stride-80 layout) and subnormal lo;
    diagonal chunks via bf16 [v|1] (M=65). Softmax denominator accumulates
    in PSUM row 64 throughout.
  - normalize: DVE reciprocal + GPSIMD partition-broadcast + DVE multiply.
  - proj: bf16 over this core's 512 channels; host sums the 2 group
    partials per batch (bf16) and adds proj_b.

Emission order is the schedule (in-order engine queues): qkv / v-transpose
/ proj units are paced by per-unit deadlines derived from when the
attention loop first needs their outputs, keeping filler work available
through the exp-latency-bound chunk loop (including late jj blocks).

PSUM (8 banks): sc [128,1024] x2 (4) + po0/po1 [65,512] (2) + shared
512-wide ring x2 (2) for qkv-acc / v-transpose / proj.
"""

import numpy as np
import ml_dtypes

import concourse.mybir as mybir
import concourse.tile as tile
from concourse import bacc
from concourse.bass_utils import run_bass_kernel_spmd
from concourse.masks import make_identity

F32 = mybir.dt.float32
BF16 = mybir.dt.bfloat16
FP8 = mybir.dt.float8e4
ACT_F = mybir.ActivationFunctionType
ALU = mybir.AluOpType
DR = mybir.MatmulPerfMode.DoubleRow

B, T, C, H, HD = 4, 2048, 1024, 16, 64
NCORES = 8
P = 128
CH = 512           # channels per core (8 heads)
NHP = 4            # head-pairs per core
NPAIR = 6          # off-diag chunk pairs per hp (chunks 0..11)
SCL = 0.125

E4 = ml_dtypes.float8_e4m3
BF = ml_dtypes.bfloat16

_CACHED = {}

# attn yield-index layout: jj block starts / per-hp strides
_S = [0, 20, 56, 108]
_STRIDE = [5, 9, 13, 17]
_NYIELD = 176


def _build():
    nc = bacc.Bacc("TRN2", target_bir_lowering=False, debug=False)

    xb_d = nc.dram_tensor("xb", [C, T], BF16, kind="ExternalInput").ap()
    wq_d = nc.dram_tensor("wq", [C, CH], BF16, kind="ExternalInput").ap()
    wk_d = nc.dram_tensor("wk", [C, CH], BF16, kind="ExternalInput").ap()
    wv_d = nc.dram_tensor("wv", [C, CH], BF16, kind="ExternalInput").ap()
    pw_d = nc.dram_tensor("pw", [CH, C], BF16, kind="ExternalInput").ap()
    qb_d = nc.dram_tensor("qb", [CH, 1], F32, kind="ExternalInput").ap()
    kb_d = nc.dram_tensor("kb", [CH, 1], F32, kind="ExternalInput").ap()
    vb_d = nc.dram_tensor("vb", [CH, 1], F32, kind="ExternalInput").ap()
    yT_d = nc.dram_tensor("yT", [C, T], BF16, kind="ExternalOutput").ap()

    xb_r = xb_d.rearrange("(k p) t -> p k t", p=P)

    with tile.TileContext(nc) as tc:
        with (
            tc.tile_pool(name="const", bufs=1) as pc,
            tc.tile_pool(name="xs", bufs=2) as px,
            tc.tile_pool(name="big", bufs=1) as pb,
            tc.tile_pool(name="exp", bufs=3) as pe,
            tc.tile_pool(name="small", bufs=4) as psm,
            tc.tile_pool(name="yev", bufs=6) as py,
            tc.tile_pool(name="ps", bufs=1, space="PSUM") as pp,
        ):
            # ---- x block 0 + v weights first (halved DMAs so the first
            # qkv chain can start after half the bytes land)
            xbt0 = px.tile([P, 8, 512], BF16, tag="xbt", name="xbt_0")
            wvt = pc.tile([P, 8, CH], BF16, tag="wvt")
            wv_r = wv_d.rearrange("(k p) c -> p k c", p=P)
            nc.sync.dma_start(xbt0[:, 0:4, :], xb_r[:, 0:4, 0:512])
            nc.sync.dma_start(wvt[:, 0:4, :], wv_r[:, 0:4, :])
            nc.sync.dma_start(xbt0[:, 4:8, :], xb_r[:, 4:8, 0:512])
            nc.sync.dma_start(wvt[:, 4:8, :], wv_r[:, 4:8, :])
            wkt = pc.tile([P, 8, CH], BF16, tag="wkt")
            nc.sync.dma_start(wkt[:], wk_d.rearrange("(k p) c -> p k c", p=P))
            wqt = pc.tile([P, 8, CH], BF16, tag="wqt")
            nc.sync.dma_start(wqt[:], wq_d.rearrange("(k p) c -> p k c", p=P))
            qbt = pc.tile([P, NHP, 1], F32, tag="qbt")
            nc.sync.dma_start(qbt[:], qb_d.rearrange("(a p) o -> p a o", p=P))
            kbt = pc.tile([P, NHP, 1], F32, tag="kbt")
            nc.sync.dma_start(kbt[:], kb_d.rearrange("(a p) o -> p a o", p=P))
            vbt = pc.tile([P, NHP, 1], F32, tag="vbt")
            nc.sync.dma_start(vbt[:], vb_d.rearrange("(a p) o -> p a o", p=P))
            pwt = pc.tile([P, NHP, C], BF16, tag="pwt")
            nc.sync.dma_start(pwt[:], pw_d.rearrange("(a p) c -> p a c", p=P))

            # ---- constants ----
            identb = pc.tile([P, P], BF16, tag="identb")
            make_identity(nc, identb)
            ones8 = pc.tile([P, 1], FP8, tag="ones8")
            nc.vector.memset(ones8[:], 1.0)
            onesb = pc.tile([P, 1], BF16, tag="onesb")
            nc.vector.memset(onesb[:], 1.0)

            # ---- per-core state ----
            qTb = pb.tile([P, NHP, T], BF16, tag="qTb")
            kTb = pb.tile([P, NHP, T], BF16, tag="kTb")
            vT = pb.tile([P, NHP, T], BF16, tag="vT")
            attn = pb.tile([P, NHP, T], BF16, tag="attn")
            vsb_t = pb.tile([P, NHP * 2 * 16 * 65], BF16, tag="vsb")
            vsb = vsb_t.rearrange("p (h l ck c) -> p h l ck c",
                                  h=NHP, l=2, ck=16)
            vhi_t = pb.tile([P, NHP * 2 * NPAIR * 2 * 80], FP8, tag="vhi")
            vhi = vhi_t.rearrange("p (h l pr kt c) -> p h l pr kt c",
                                  h=NHP, l=2, pr=NPAIR, kt=2)
            vlo_t = pb.tile([P, NHP * 2 * NPAIR * 2 * 64], FP8, tag="vlo")
            vlo = vlo_t.rearrange("p (h l pr kt c) -> p h l pr kt c",
                                  h=NHP, l=2, pr=NPAIR, kt=2)
            nc.vector.tensor_copy(
                vsb_t.rearrange("p (s c) -> p s c", c=65)[:, :, 64:65],
                onesb[:, None, 0:1].broadcast_to((P, NHP * 2 * 16, 1)))
            nc.vector.tensor_copy(
                vhi_t.rearrange("p (s c) -> p s c", c=80)[:, :, 64:65],
                ones8[:, None, 0:1].broadcast_to((P, NHP * 2 * NPAIR * 2, 1)))

            def qkv_gen():
                """48 units: per nbq, 4 v then (k,q) per hp."""
                xnext = xbt0
                for nbq in range(4):
                    xbt = xnext
                    if nbq + 1 < 4:
                        blk1 = slice((nbq + 1) * 512, (nbq + 2) * 512)
                        xnext = px.tile([P, 8, 512], BF16, tag="xbt",
                                        name=f"xbt_{nbq + 1}")
                        nc.sync.dma_start(xnext[:], xb_r[:, :, blk1])
                    blk = slice(nbq * 512, (nbq + 1) * 512)

                    def chain(wt, m):
                        acc = pp.tile([P, 512], F32, tag="w512", bufs=2,
                                      name=f"acc_{nbq}_{m}")
                        for kt in range(8):
                            nc.tensor.matmul(
                                acc[:], wt[:, kt, P * m:P * (m + 1)],
                                xbt[:, kt, :], start=(kt == 0), stop=(kt == 7))
                        return acc

                    for m in range(NHP):
                        acc = chain(wvt, m)
                        nc.vector.tensor_scalar(
                            out=vT[:, m, blk], in0=acc[:],
                            scalar1=vbt[:, m, 0:1], scalar2=None, op0=ALU.add)
                        yield
                    for m in range(NHP):
                        acc = chain(wkt, m)
                        nc.vector.tensor_scalar(
                            out=kTb[:, m, blk], in0=acc[:],
                            scalar1=kbt[:, m, 0:1], scalar2=None, op0=ALU.add)
                        yield
                        acc = chain(wqt, m)
                        nc.vector.tensor_scalar(
                            out=qTb[:, m, blk], in0=acc[:],
                            scalar1=qbt[:, m, 0:1], scalar2=None, op0=ALU.add)
                        yield

            def vsb_gen():
                """64 units: (nbq, hp, ck)."""
                for nbq in range(4):
                    for hp in range(NHP):
                        for ck in range(4 * nbq, 4 * nbq + 4):
                            tp = pp.tile([P, P], BF16, tag="w512", bufs=2,
                                         name=f"tp_{hp}_{ck}")
                            nc.tensor.transpose(
                                tp[:], vT[:, hp, ck * P:(ck + 1) * P],
                                identb[:])
                            src = tp[:].rearrange("p (l c) -> p l c", l=2)
                            dstb = vsb[:, hp, :, ck, 0:64]
                            nc.vector.tensor_copy(dstb, src)
                            if ck < 2 * NPAIR:
                                pair, kt = ck // 2, ck % 2
                                dst8 = vhi[:, hp, :, pair, kt, 0:64]
                                nc.vector.tensor_copy(dst8, dstb)
                                nc.vector.tensor_tensor(
                                    out=vlo[:, hp, :, pair, kt, :],
                                    in0=dstb, in1=dst8, op=ALU.subtract)
                            yield

            def attn_gen(ready):
                for jj in range(4):
                    tqa = jj * 512
                    for hp in range(NHP):
                        po = [pp.tile([65, 512], F32, tag=f"po{l}", bufs=1,
                                      name=f"po_{jj}_{hp}_{l}")
                              for l in range(2)]
                        nch = 4 * jj + 4
                        first = {"f": True}
                        pend = []

                        def emit_vmm(item, po=po, hp=hp, nch=nch, first=first):
                            kind, a, ex = item
                            st = first["f"]
                            first["f"] = False
                            if kind == "pair":
                                for l in range(2):
                                    rhs = ex[:, :, 512 * l:512 * (l + 1)]
                                    nc.tensor.matmul(
                                        po[l][:, :],
                                        vhi[:, hp, l, a, :, 0:65], rhs,
                                        start=st, stop=False,
                                        perf_mode=DR, skip_group_check=True)
                                    nc.tensor.matmul(
                                        po[l][0:64, :],
                                        vlo[:, hp, l, a, :, :], rhs,
                                        start=False, stop=False,
                                        perf_mode=DR, skip_group_check=True)
                            else:
                                ck, w = a
                                last = ck == nch - 1
                                for l in range(2):
                                    nc.tensor.matmul(
                                        po[l][:, w:512],
                                        vsb[:, hp, l, ck, 0:65],
                                        ex[:, 512 * l + w:512 * (l + 1)],
                                        start=st, stop=last,
                                        skip_group_check=True)

                        ex8 = None
                        for ck in range(nch):
                            w = max(0, 128 * (ck - 4 * jj))
                            diag = ck >= 4 * jj
                            sc = pp.tile([P, 1024], F32, tag="sc", bufs=2,
                                         name=f"sc_{jj}_{hp}_{ck}")
                            for l in range(2):
                                nc.tensor.matmul(
                                    sc[:, 512 * l + w:512 * (l + 1)],
                                    kTb[64 * l:64 * l + 64, hp,
                                        ck * P:(ck + 1) * P],
                                    qTb[64 * l:64 * l + 64, hp,
                                        tqa + w:tqa + 512],
                                    start=True, stop=True,
                                    tile_position=(64 * l, 0))
                            if diag:
                                exb = pe.tile([P, 1024], BF16, tag="exb",
                                              bufs=3,
                                              name=f"exb_{jj}_{hp}_{ck}")
                                nc.scalar.activation(
                                    exb[:, w:1024], sc[:, w:1024],
                                    ACT_F.Exp, scale=SCL)
                                exv = exb[:].rearrange(
                                    "p (l t) -> p l t", l=2)[:, :, w:w + P]
                                nc.gpsimd.affine_select(
                                    out=exv, in_=exv,
                                    compare_op=ALU.is_ge, fill=0.0, base=0,
                                    pattern=[[0, 2], [1, P]],
                                    channel_multiplier=-1)
                                pend.append(("diag", (ck, w), exb))
                            else:
                                par = ck % 2
                                if par == 0:
                                    ex8 = pe.tile([P, 2, 1024], FP8,
                                                  tag="ex8", bufs=3,
                                                  name=f"ex8_{jj}_{hp}_{ck}")
                                nc.scalar.activation(
                                    ex8[:, par, :], sc[:, :],
                                    ACT_F.Exp, scale=SCL)
                                if par == 1:
                                    pend.append(("pair", ck // 2, ex8))
                            if len(pend) > 1:
                                emit_vmm(pend.pop(0))
                            yield
                        while pend:
                            emit_vmm(pend.pop(0))
                        rcs, rbs = [], []
                        for l in range(2):
                            rc = psm.tile([1, 512], F32, tag="rc", bufs=4,
                                          name=f"rc_{jj}_{hp}_{l}")
                            nc.vector.reciprocal(rc[:], po[l][64:65, :])
                            rcs.append(rc)
                        for l in range(2):
                            rb = psm.tile([64, 512], F32, tag="rb", bufs=4,
                                          name=f"rb_{jj}_{hp}_{l}")
                            nc.gpsimd.partition_broadcast(rb[:], rcs[l][0:1, :])
                            rbs.append(rb)
                        for l in range(2):
                            nc.vector.tensor_tensor(
                                out=attn[64 * l:64 * l + 64, hp,
                                         tqa:tqa + 512],
                                in0=po[l][0:64, :], in1=rbs[l][:], op=ALU.mult)
                        yield
                    ready.append(jj)

            def proj_gen(ready):
                done = 0
                while done < 4:
                    if not ready:
                        yield
                        continue
                    nb = ready.pop(0)
                    blk = slice(nb * 512, (nb + 1) * 512)
                    for ob in range(8):
                        pj = pp.tile([P, 512], F32, tag="w512", bufs=2,
                                     name=f"pj_{nb}_{ob}")
                        for hp in range(NHP):
                            nc.tensor.matmul(
                                pj[:], pwt[:, hp, ob * P:(ob + 1) * P],
                                attn[:, hp, blk],
                                start=(hp == 0), stop=(hp == 3))
                        ysb = py.tile([P, 512], BF16, tag="ysb",
                                      name=f"ysb_{nb}_{ob}")
                        nc.vector.tensor_copy(ysb[:], pj[:])
                        nc.sync.dma_start(
                            yT_d[ob * P:(ob + 1) * P, blk], ysb[:])
                        yield
                    done += 1

            # ---- deadline-based pacing --------------------------------
            # qkv unit u (0-based, 12 per nbq: v x4 then k,q per hp):
            qk_dl = []
            for nbq in range(4):
                for m in range(NHP):       # v tiles: before vsb group nbq
                    qk_dl.append(max(0, _S[nbq] - 8))
                for m in range(NHP):       # k,q per hp: before attn(jj=nbq, hp=m)
                    d = _S[nbq] + m * _STRIDE[nbq]
                    qk_dl.extend([d, d])
            vs_dl = []
            for nbq in range(4):
                for hp in range(NHP):
                    d = _S[nbq] + hp * _STRIDE[nbq]
                    vs_dl.extend([d] * 4)

            LOOKAHEAD = 7

            def target(dl, i):
                t = 0
                for j, d in enumerate(dl):
                    if d <= i + LOOKAHEAD:
                        t = j + 1
                return t

            qk = qkv_gen()
            vs = vsb_gen()
            ready = []
            at = attn_gen(ready)
            pj = proj_gen(ready)

            nqk, nvs = [0], [0]
            _DONE = object()

            def adv(gen, cnt, tgt):
                while cnt[0] < tgt:
                    if next(gen, _DONE) is _DONE:
                        break
                    cnt[0] += 1

            adv(qk, nqk, 12)   # nbq0 fully
            adv(vs, nvs, 4)

            i = 0
            for _ in at:
                adv(qk, nqk, target(qk_dl, i))
                adv(vs, nvs, target(vs_dl, i))
                # proj filler weighted into the exp-bound late jj blocks
                if ready and i >= 40 and i % 3 == 0:
                    next(pj, None)
                i += 1
            for gen in (qk, vs, pj):
                for _ in gen:
                    pass

    nc.compile()
    return nc


def get_nc():
    if "nc" not in _CACHED:
        _CACHED["nc"] = _build()
    return _CACHED["nc"]


def kernel(x, qkv_w, qkv_b, proj_w, proj_b):
    x = np.asarray(x, dtype=np.float32)
    qkv_w = np.asarray(qkv_w, dtype=np.float32)
    qkv_b = np.asarray(qkv_b, dtype=np.float32)
    proj_w = np.asarray(proj_w, dtype=np.float32)
    proj_b = np.asarray(proj_b, dtype=np.float32)

    in_maps = []
    for c in range(NCORES):
        b, grp = c // 2, c % 2
        cs = slice(CH * grp, CH * (grp + 1))
        xT = np.ascontiguousarray(x[b].T)              # [C, T]
        in_maps.append({
            "xb": xT.astype(BF),
            "wq": np.ascontiguousarray(qkv_w[cs, :].T).astype(BF),
            "wk": np.ascontiguousarray(qkv_w[C:][cs, :].T).astype(BF),
            "wv": np.ascontiguousarray(qkv_w[2 * C:][cs, :].T).astype(BF),
            "pw": np.ascontiguousarray(proj_w[:, cs].T).astype(BF),
            "qb": qkv_b[cs].reshape(CH, 1).astype(np.float32),
            "kb": qkv_b[C:][cs].reshape(CH, 1).astype(np.float32),
            "vb": qkv_b[2 * C:][cs].reshape(CH, 1).astype(np.float32),
        })

    nc = get_nc()
    res = run_bass_kernel_spmd(nc, in_maps, list(range(NCORES)))

    y = np.empty((B, T, C), dtype=np.float32)
    pbf = proj_b.astype(np.float64)[None, :]
    for b in range(B):
        yT = (res.results[2 * b]["yT"].astype(np.float64)
              + res.results[2 * b + 1]["yT"].astype(np.float64))
        y[b] = (yT.T + pbf).astype(np.float32)
    return y


# revision 12
# speedup vs baseline: 1.1440x; 1.0094x over previous
"""Causal self-attention for TRN2, 8 NeuronCores, DP(batch=4) x TP(2 head-groups).

Problem (hardcoded): B=4, T=2048, C=1024, H=16 heads, hd=64.
  qkv = x @ qkv_w.T + qkv_b ; per-head causal softmax(q k^T / 8) @ v ; out @ proj_w.T + proj_b

Core (b, g) handles batch b and head-group g (8 heads = 512 channels).
All inputs stream as bf16; the far-past attention path runs fp8e4
DoubleRow (2x PE rate):

  - qkv projections: bf16 matmuls, q/k evicted via ACT Identity+bias,
    v via DVE.
  - attention per head-pair hp in the transposed domain (2 heads packed in
    128 partitions via tile_position rows): scoresT[tk,tq] bf16, K=64.
  - exp via ACT: off-diagonal chunks -> fp8 pair-tiles (for DoubleRow),
    diagonal chunks -> bf16; causal mask by zeroing the upper triangle of
    diagonal ex blocks on GPSIMD (affine_select).
  - attn@v: off-diagonal chunk PAIRS via fp8 DoubleRow (K=256/pass) with v
    split into fp8 hi (+ones col, M=65, stride-80 layout) and subnormal lo;
    diagonal chunks via bf16 [v|1] (M=65). Softmax denominator accumulates
    in PSUM row 64 throughout.
  - normalize: DVE reciprocal + GPSIMD partition-broadcast + DVE multiply.
  - proj: bf16 over this core's 512 channels; host sums the 2 group
    partials per batch (bf16) and adds proj_b.

Emission order is the schedule (in-order engine queues): qkv / v-transpose
/ proj units are paced by per-unit deadlines derived from when the
attention loop first needs their outputs, keeping filler work available
through the exp-latency-bound chunk loop (including late jj blocks).

PSUM (8 banks): sc [128,1024] x2 (4) + po0/po1 [65,512] (2) + shared
512-wide ring x2 (2) for qkv-acc / v-transpose / proj.
"""

import numpy as np
import ml_dtypes

import concourse.mybir as mybir
import concourse.tile as tile
from concourse import bacc
from concourse.bass_utils import run_bass_kernel_spmd
from concourse.masks import make_identity

F32 = mybir.dt.float32
BF16 = mybir.dt.bfloat16
FP8 = mybir.dt.float8e4
ACT_F = mybir.ActivationFunctionType
ALU = mybir.AluOpType
DR = mybir.MatmulPerfMode.DoubleRow

B, T, C, H, HD = 4, 2048, 1024, 16, 64
NCORES = 8
P = 128
CH = 512           # channels per core (8 heads)
NHP = 4            # head-pairs per core
NPAIR = 6          # off-diag chunk pairs per hp (chunks 0..11)
SCL = 0.125

E4 = ml_dtypes.float8_e4m3
BF = ml_dtypes.bfloat16

_CACHED = {}

# attn yield-index layout: jj block starts / per-hp strides
_S = [0, 24, 64, 120]
_STRIDE = [6, 10, 14, 18]
_NYIELD = 192


def _build():
    nc = bacc.Bacc("TRN2", target_bir_lowering=False, debug=False)

    xb_d = nc.dram_tensor("xb", [C, T], BF16, kind="ExternalInput").ap()
    wq_d = nc.dram_tensor("wq", [C, CH], BF16, kind="ExternalInput").ap()
    wk_d = nc.dram_tensor("wk", [C, CH], BF16, kind="ExternalInput").ap()
    wv_d = nc.dram_tensor("wv", [C, CH], BF16, kind="ExternalInput").ap()
    pw_d = nc.dram_tensor("pw", [CH, C], BF16, kind="ExternalInput").ap()
    qb_d = nc.dram_tensor("qb", [CH, 1], F32, kind="ExternalInput").ap()
    kb_d = nc.dram_tensor("kb", [CH, 1], F32, kind="ExternalInput").ap()
    vb_d = nc.dram_tensor("vb", [CH, 1], F32, kind="ExternalInput").ap()
    yT_d = nc.dram_tensor("yT", [C, T], BF16, kind="ExternalOutput").ap()

    xb_r = xb_d.rearrange("(k p) t -> p k t", p=P)

    with tile.TileContext(nc) as tc:
        with (
            tc.tile_pool(name="const", bufs=1) as pc,
            tc.tile_pool(name="xs", bufs=2) as px,
            tc.tile_pool(name="big", bufs=1) as pb,
            tc.tile_pool(name="exp", bufs=3) as pe,
            tc.tile_pool(name="small", bufs=4) as psm,
            tc.tile_pool(name="yev", bufs=6) as py,
            tc.tile_pool(name="ps", bufs=1, space="PSUM") as pp,
        ):
            # ---- x block 0 + v weights first (halved DMAs so the first
            # qkv chain can start after half the bytes land)
            xbt0 = px.tile([P, 8, 512], BF16, tag="xbt", name="xbt_0")
            wvt = pc.tile([P, 8, CH], BF16, tag="wvt")
            wv_r = wv_d.rearrange("(k p) c -> p k c", p=P)
            nc.sync.dma_start(xbt0[:, 0:4, :], xb_r[:, 0:4, 0:512])
            nc.sync.dma_start(wvt[:, 0:4, :], wv_r[:, 0:4, :])
            nc.sync.dma_start(xbt0[:, 4:8, :], xb_r[:, 4:8, 0:512])
            nc.sync.dma_start(wvt[:, 4:8, :], wv_r[:, 4:8, :])
            wkt = pc.tile([P, 8, CH], BF16, tag="wkt")
            nc.sync.dma_start(wkt[:], wk_d.rearrange("(k p) c -> p k c", p=P))
            wqt = pc.tile([P, 8, CH], BF16, tag="wqt")
            nc.sync.dma_start(wqt[:], wq_d.rearrange("(k p) c -> p k c", p=P))
            qbt = pc.tile([P, NHP, 1], F32, tag="qbt")
            nc.sync.dma_start(qbt[:], qb_d.rearrange("(a p) o -> p a o", p=P))
            kbt = pc.tile([P, NHP, 1], F32, tag="kbt")
            nc.sync.dma_start(kbt[:], kb_d.rearrange("(a p) o -> p a o", p=P))
            vbt = pc.tile([P, NHP, 1], F32, tag="vbt")
            nc.sync.dma_start(vbt[:], vb_d.rearrange("(a p) o -> p a o", p=P))
            pwt = pc.tile([P, NHP, C], BF16, tag="pwt")
            nc.sync.dma_start(pwt[:], pw_d.rearrange("(a p) c -> p a c", p=P))

            # ---- constants ----
            identb = pc.tile([P, P], BF16, tag="identb")
            make_identity(nc, identb)
            ones8 = pc.tile([P, 1], FP8, tag="ones8")
            nc.vector.memset(ones8[:], 1.0)
            onesb = pc.tile([P, 1], BF16, tag="onesb")
            nc.vector.memset(onesb[:], 1.0)

            # ---- per-core state ----
            qTb = pb.tile([P, NHP, T], BF16, tag="qTb")
            kTb = pb.tile([P, NHP, T], BF16, tag="kTb")
            vT = pb.tile([P, NHP, T], BF16, tag="vT")
            attn = pb.tile([P, NHP, T], BF16, tag="attn")
            vsb_t = pb.tile([P, NHP * 2 * 16 * 65], BF16, tag="vsb")
            vsb = vsb_t.rearrange("p (h l ck c) -> p h l ck c",
                                  h=NHP, l=2, ck=16)
            vhi_t = pb.tile([P, NHP * 2 * NPAIR * 2 * 80], FP8, tag="vhi")
            vhi = vhi_t.rearrange("p (h l pr kt c) -> p h l pr kt c",
                                  h=NHP, l=2, pr=NPAIR, kt=2)
            vlo_t = pb.tile([P, NHP * 2 * NPAIR * 2 * 64], FP8, tag="vlo")
            vlo = vlo_t.rearrange("p (h l pr kt c) -> p h l pr kt c",
                                  h=NHP, l=2, pr=NPAIR, kt=2)
            nc.vector.tensor_copy(
                vsb_t.rearrange("p (s c) -> p s c", c=65)[:, :, 64:65],
                onesb[:, None, 0:1].broadcast_to((P, NHP * 2 * 16, 1)))
            nc.vector.tensor_copy(
                vhi_t.rearrange("p (s c) -> p s c", c=80)[:, :, 64:65],
                ones8[:, None, 0:1].broadcast_to((P, NHP * 2 * NPAIR * 2, 1)))

            def qkv_gen():
                """48 units: per nbq, 4 v then (k,q) per hp."""
                xnext = xbt0
                for nbq in range(4):
                    xbt = xnext
                    if nbq + 1 < 4:
                        blk1 = slice((nbq + 1) * 512, (nbq + 2) * 512)
                        xnext = px.tile([P, 8, 512], BF16, tag="xbt",
                                        name=f"xbt_{nbq + 1}")
                        nc.sync.dma_start(xnext[:], xb_r[:, :, blk1])
                    blk = slice(nbq * 512, (nbq + 1) * 512)

                    def chain(wt, m):
                        acc = pp.tile([P, 512], F32, tag="w512", bufs=2,
                                      name=f"acc_{nbq}_{m}")
                        for kt in range(8):
                            nc.tensor.matmul(
                                acc[:], wt[:, kt, P * m:P * (m + 1)],
                                xbt[:, kt, :], start=(kt == 0), stop=(kt == 7))
                        return acc

                    for m in range(NHP):
                        acc = chain(wvt, m)
                        nc.vector.tensor_scalar(
                            out=vT[:, m, blk], in0=acc[:],
                            scalar1=vbt[:, m, 0:1], scalar2=None, op0=ALU.add)
                        yield
                    for m in range(NHP):
                        acc = chain(wkt, m)
                        nc.vector.tensor_scalar(
                            out=kTb[:, m, blk], in0=acc[:],
                            scalar1=kbt[:, m, 0:1], scalar2=None, op0=ALU.add)
                        yield
                        acc = chain(wqt, m)
                        nc.vector.tensor_scalar(
                            out=qTb[:, m, blk], in0=acc[:],
                            scalar1=qbt[:, m, 0:1], scalar2=None, op0=ALU.add)
                        yield

            def vsb_gen():
                """64 units: (nbq, hp, ck)."""
                for nbq in range(4):
                    for hp in range(NHP):
                        for ck in range(4 * nbq, 4 * nbq + 4):
                            tp = pp.tile([P, P], BF16, tag="w512", bufs=2,
                                         name=f"tp_{hp}_{ck}")
                            nc.tensor.transpose(
                                tp[:], vT[:, hp, ck * P:(ck + 1) * P],
                                identb[:])
                            src = tp[:].rearrange("p (l c) -> p l c", l=2)
                            dstb = vsb[:, hp, :, ck, 0:64]
                            nc.vector.tensor_copy(dstb, src)
                            if ck < 2 * NPAIR:
                                pair, kt = ck // 2, ck % 2
                                dst8 = vhi[:, hp, :, pair, kt, 0:64]
                                nc.vector.tensor_copy(dst8, dstb)
                                nc.vector.tensor_tensor(
                                    out=vlo[:, hp, :, pair, kt, :],
                                    in0=dstb, in1=dst8, op=ALU.subtract)
                            yield

            def attn_gen(ready):
                for jj in range(4):
                    tqa = jj * 512
                    for hp in range(NHP):
                        po = [pp.tile([65, 512], F32, tag=f"po{l}", bufs=1,
                                      name=f"po_{jj}_{hp}_{l}")
                              for l in range(2)]
                        nch = 4 * jj + 4
                        first = {"f": True}
                        pend = []

                        def emit_vmm(item, po=po, hp=hp, nch=nch, first=first):
                            kind, a, ex = item
                            st = first["f"]
                            first["f"] = False
                            if kind == "pair":
                                for l in range(2):
                                    rhs = ex[:, :, 512 * l:512 * (l + 1)]
                                    nc.tensor.matmul(
                                        po[l][:, :],
                                        vhi[:, hp, l, a, :, 0:65], rhs,
                                        start=st, stop=False,
                                        perf_mode=DR, skip_group_check=True)
                                    nc.tensor.matmul(
                                        po[l][0:64, :],
                                        vlo[:, hp, l, a, :, :], rhs,
                                        start=False, stop=False,
                                        perf_mode=DR, skip_group_check=True)
                            else:
                                ck, w = a
                                last = ck == nch - 1
                                for l in range(2):
                                    nc.tensor.matmul(
                                        po[l][:, w:512],
                                        vsb[:, hp, l, ck, 0:65],
                                        ex[:, 512 * l + w:512 * (l + 1)],
                                        start=st, stop=last,
                                        skip_group_check=True)

                        ex8 = None
                        for ck in range(nch):
                            w = max(0, 128 * (ck - 4 * jj))
                            diag = ck >= 4 * jj
                            sc = pp.tile([P, 1024], F32, tag="sc", bufs=2,
                                         name=f"sc_{jj}_{hp}_{ck}")
                            for l in range(2):
                                nc.tensor.matmul(
                                    sc[:, 512 * l + w:512 * (l + 1)],
                                    kTb[64 * l:64 * l + 64, hp,
                                        ck * P:(ck + 1) * P],
                                    qTb[64 * l:64 * l + 64, hp,
                                        tqa + w:tqa + 512],
                                    start=True, stop=True,
                                    tile_position=(64 * l, 0))
                            if diag:
                                exb = pe.tile([P, 1024], BF16, tag="exb",
                                              bufs=3,
                                              name=f"exb_{jj}_{hp}_{ck}")
                                nc.scalar.activation(
                                    exb[:, w:1024], sc[:, w:1024],
                                    ACT_F.Exp, scale=SCL)
                                exv = exb[:].rearrange(
                                    "p (l t) -> p l t", l=2)[:, :, w:w + P]
                                nc.gpsimd.affine_select(
                                    out=exv, in_=exv,
                                    compare_op=ALU.is_ge, fill=0.0, base=0,
                                    pattern=[[0, 2], [1, P]],
                                    channel_multiplier=-1)
                                pend.append(("diag", (ck, w), exb))
                            else:
                                par = ck % 2
                                if par == 0:
                                    ex8 = pe.tile([P, 2, 1024], FP8,
                                                  tag="ex8", bufs=3,
                                                  name=f"ex8_{jj}_{hp}_{ck}")
                                nc.scalar.activation(
                                    ex8[:, par, :], sc[:, :],
                                    ACT_F.Exp, scale=SCL)
                                if par == 1:
                                    pend.append(("pair", ck // 2, ex8))
                            if len(pend) > 1:
                                emit_vmm(pend.pop(0))
                            yield
                        while pend:
                            emit_vmm(pend.pop(0))
                            yield    # filler slot between flush items
                        rcs, rbs = [], []
                        for l in range(2):
                            rc = psm.tile([1, 512], F32, tag="rc", bufs=4,
                                          name=f"rc_{jj}_{hp}_{l}")
                            nc.vector.reciprocal(rc[:], po[l][64:65, :])
                            rcs.append(rc)
                        for l in range(2):
                            rb = psm.tile([64, 512], F32, tag="rb", bufs=4,
                                          name=f"rb_{jj}_{hp}_{l}")
                            nc.gpsimd.partition_broadcast(rb[:], rcs[l][0:1, :])
                            rbs.append(rb)
                        for l in range(2):
                            nc.vector.tensor_tensor(
                                out=attn[64 * l:64 * l + 64, hp,
                                         tqa:tqa + 512],
                                in0=po[l][0:64, :], in1=rbs[l][:], op=ALU.mult)
                        yield
                    ready.append(jj)

            def proj_gen(ready):
                done = 0
                while done < 4:
                    if not ready:
                        yield
                        continue
                    nb = ready.pop(0)
                    blk = slice(nb * 512, (nb + 1) * 512)
                    for ob in range(8):
                        pj = pp.tile([P, 512], F32, tag="w512", bufs=2,
                                     name=f"pj_{nb}_{ob}")
                        for hp in range(NHP):
                            nc.tensor.matmul(
                                pj[:], pwt[:, hp, ob * P:(ob + 1) * P],
                                attn[:, hp, blk],
                                start=(hp == 0), stop=(hp == 3))
                        ysb = py.tile([P, 512], BF16, tag="ysb",
                                      name=f"ysb_{nb}_{ob}")
                        nc.vector.tensor_copy(ysb[:], pj[:])
                        nc.sync.dma_start(
                            yT_d[ob * P:(ob + 1) * P, blk], ysb[:])
                        yield
                    done += 1

            # ---- deadline-based pacing --------------------------------
            # qkv unit u (0-based, 12 per nbq: v x4 then k,q per hp):
            qk_dl = []
            for nbq in range(4):
                for m in range(NHP):       # v tiles: before vsb group nbq
                    qk_dl.append(max(0, _S[nbq] - 8))
                for m in range(NHP):       # k,q per hp: before attn(jj=nbq, hp=m)
                    d = _S[nbq] + m * _STRIDE[nbq]
                    qk_dl.extend([d, d])
            vs_dl = []
            for nbq in range(4):
                for hp in range(NHP):
                    d = _S[nbq] + hp * _STRIDE[nbq]
                    vs_dl.extend([d] * 4)

            LOOKAHEAD = 7

            def target(dl, i):
                t = 0
                for j, d in enumerate(dl):
                    if d <= i + LOOKAHEAD:
                        t = j + 1
                return t

            qk = qkv_gen()
            vs = vsb_gen()
            ready = []
            at = attn_gen(ready)
            pj = proj_gen(ready)

            nqk, nvs = [0], [0]
            _DONE = object()

            def adv(gen, cnt, tgt, cap=None):
                n = 0
                while cnt[0] < tgt:
                    if cap is not None and n >= cap:
                        break
                    if next(gen, _DONE) is _DONE:
                        break
                    cnt[0] += 1
                    n += 1

            adv(qk, nqk, 12)   # nbq0 fully
            adv(vs, nvs, 4)

            i = 0
            for _ in at:
                adv(qk, nqk, target(qk_dl, i), cap=1)
                adv(vs, nvs, target(vs_dl, i), cap=2)
                # proj filler weighted into the exp-bound late jj blocks
                if ready and i >= 44 and i % 3 == 0:
                    next(pj, None)
                i += 1
            for gen in (qk, vs, pj):
                for _ in gen:
                    pass

    nc.compile()
    return nc


def get_nc():
    if "nc" not in _CACHED:
        _CACHED["nc"] = _build()
    return _CACHED["nc"]


def kernel(x, qkv_w, qkv_b, proj_w, proj_b):
    x = np.asarray(x, dtype=np.float32)
    qkv_w = np.asarray(qkv_w, dtype=np.float32)
    qkv_b = np.asarray(qkv_b, dtype=np.float32)
    proj_w = np.asarray(proj_w, dtype=np.float32)
    proj_b = np.asarray(proj_b, dtype=np.float32)

    in_maps = []
    for c in range(NCORES):
        b, grp = c // 2, c % 2
        cs = slice(CH * grp, CH * (grp + 1))
        xT = np.ascontiguousarray(x[b].T)              # [C, T]
        in_maps.append({
            "xb": xT.astype(BF),
            "wq": np.ascontiguousarray(qkv_w[cs, :].T).astype(BF),
            "wk": np.ascontiguousarray(qkv_w[C:][cs, :].T).astype(BF),
            "wv": np.ascontiguousarray(qkv_w[2 * C:][cs, :].T).astype(BF),
            "pw": np.ascontiguousarray(proj_w[:, cs].T).astype(BF),
            "qb": qkv_b[cs].reshape(CH, 1).astype(np.float32),
            "kb": qkv_b[C:][cs].reshape(CH, 1).astype(np.float32),
            "vb": qkv_b[2 * C:][cs].reshape(CH, 1).astype(np.float32),
        })

    nc = get_nc()
    res = run_bass_kernel_spmd(nc, in_maps, list(range(NCORES)))

    y = np.empty((B, T, C), dtype=np.float32)
    pbf = proj_b.astype(np.float64)[None, :]
    for b in range(B):
        yT = (res.results[2 * b]["yT"].astype(np.float64)
              + res.results[2 * b + 1]["yT"].astype(np.float64))
        y[b] = (yT.T + pbf).astype(np.float32)
    return y
